# revision 6
# baseline (speedup 1.0000x reference)
"""Trainium2 Bass kernel for BalancedFrequencyAttention.

Math: the reference does DCT(W) -> frequency split/mix -> IDCT -> GAP -> tiny
SE gate -> x * att.  Everything from x to gap is linear, so gap[b,c] ==
sum_{h,n} x[b,c,h,n] * G[h,n] for a fixed matrix G.  G turns out to have only
4 distinct rows: one pattern for h < 80 and a period-3 family for h >= 80.
So the kernel is a memory-bound two-pass streaming kernel:
  pass 1: gap[c] = sum over tiles of reduce(x_tile * G_tile)   (VectorE TTR)
  MLP:    att = sigmoid(w2 @ (w1 @ gap))                        (TensorE)
  pass 2: out = x * att[c]                                      (per-partition scale)
Sharding: batch b=8 -> one sample per NeuronCore, no cross-core communication.
"""

import sys

if "/opt/trn_rl_repo" not in sys.path:
    sys.path.insert(0, "/opt/trn_rl_repo")

import numpy as np

B, C, H, W = 8, 128, 200, 480
K = W // 2          # 240: balanced-spectrum width
HS = H - W // 4     # 80: high-freq start row
N_CORES = 8

# ---- pass-1/2 tiling (rows of H) ----
LOW_TILES = [(h0, 10) for h0 in range(0, HS, 10)]        # 8 tiles, pattern period 1 row
HIGH_TILES = [(h0, 12) for h0 in range(HS, H, 12)]       # 10 tiles, pattern period 3 rows
N_CACHE = 3                                               # cache last 3 high tiles in SBUF
CACHE_START = H - N_CACHE * 12                            # rows >= 152 stay in SBUF

_nc_cache = None
_const_cache = None


def _build_constants():
    """Fold the whole DCT pipeline into per-row weight vectors (float64)."""

    def dct_mat(n):
        i = np.arange(n, dtype=np.float64)
        m = np.cos(np.pi * (2.0 * i[None, :] + 1.0) * i[:, None] / (2.0 * n)) * np.sqrt(2.0 / n)
        m[0] *= 1.0 / np.sqrt(2.0)
        return m

    D = dct_mat(W)
    D2 = dct_mat(K)
    s = D2.sum(axis=1)                     # row sums of the IDCT matrix
    t = s @ D[:K, :]                       # low-band weight profile, [W]
    alpha = 0.6 / (H * K)
    beta = 0.4 / (H * K)
    # high[b,c,h',k'] = x_dct[:, :, 80+f//400, 80+f%400], f = h'*240+k'; the
    # s-weight index (f mod 240) depends on h only through (h-80)%3.
    offs = [0, 160, 80]
    V = np.stack(
        [s[(offs[j] + np.arange(HS, W) - HS) % K] @ D[HS:W, :] for j in range(3)]
    )
    g_low = (alpha * t).astype(np.float32)                                  # [480]
    g_hi = np.concatenate([alpha * t + beta * V[j] for j in range(3)]).astype(np.float32)  # [1440]
    g_low_rep = np.ascontiguousarray(np.broadcast_to(g_low, (128, W)))
    g_hi_rep = np.ascontiguousarray(np.broadcast_to(g_hi, (128, 3 * W)))
    return g_low_rep, g_hi_rep


def _build_kernel():
    import concourse.bacc as bacc
    import concourse.tile as tile
    from concourse import mybir

    f32 = mybir.dt.float32
    nc = bacc.Bacc("TRN2", target_bir_lowering=False, debug=False, num_devices=N_CORES)

    x = nc.dram_tensor("x", [C, H, W], f32, kind="ExternalInput")
    g_low = nc.dram_tensor("g_low", [128, W], f32, kind="ExternalInput")
    g_hi = nc.dram_tensor("g_hi", [128, 3 * W], f32, kind="ExternalInput")
    w1t = nc.dram_tensor("w1t", [C, C // 4], f32, kind="ExternalInput")
    w2t = nc.dram_tensor("w2t", [C // 4, C], f32, kind="ExternalInput")
    out = nc.dram_tensor("out", [C, H, W], f32, kind="ExternalOutput")

    add = mybir.AluOpType.add
    mult = mybir.AluOpType.mult
    all_tiles = LOW_TILES + HIGH_TILES
    n_tiles = len(all_tiles)

    with tile.TileContext(nc) as tc:
        with (
            tc.tile_pool(name="stream", bufs=3) as stream,
            tc.tile_pool(name="cachep", bufs=N_CACHE) as cachep,
            tc.tile_pool(name="consts", bufs=1) as consts,
            tc.tile_pool(name="small", bufs=1) as small,
            tc.tile_pool(name="psum", bufs=1, space="PSUM") as psum,
        ):
            g_low_t = consts.tile([128, W], f32, tag="g_low")
            nc.sync.dma_start(g_low_t[:], g_low[:])
            g_hi_t = consts.tile([128, 3 * W], f32, tag="g_hi")
            nc.sync.dma_start(g_hi_t[:], g_hi[:])
            w1t_t = consts.tile([C, C // 4], f32, tag="w1t")
            nc.sync.dma_start(w1t_t[:], w1t[:])
            w2t_t = consts.tile([C // 4, C], f32, tag="w2t")
            nc.sync.dma_start(w2t_t[:], w2t[:])
            acc_hi = consts.tile([128, 12 * W], f32, tag="acc_hi")
            acc_lo = consts.tile([128, 10 * W], f32, tag="acc_lo")
            nc.vector.memset(acc_hi[:], 0.0)
            nc.vector.memset(acc_lo[:], 0.0)
            prod = consts.tile([128, 3 * W], f32, tag="prod")
            partials = small.tile([128, 2], f32, tag="partials")

            # ---- pass 1: stream x, accumulate h-class sums ----
            cached_tiles = []
            for i, (h0, rows) in enumerate(all_tiles):
                cached = h0 >= CACHE_START
                acc = acc_lo if h0 < HS else acc_hi
                pool = cachep if cached else stream
                xt = pool.tile([128, rows * W], f32, tag="cache" if cached else "xs")
                nc.sync.dma_start(
                    xt[:], x[:, h0 : h0 + rows, :].rearrange("p r w -> p (r w)")
                )
                nc.vector.tensor_add(acc[:, : rows * W], acc[:, : rows * W], xt[:])
                if cached:
                    cached_tiles.append((xt, h0, rows))

            # fold acc_hi [4, 3W] -> [3W]; rows within each 3W group are class-aligned
            nc.vector.tensor_add(acc_hi[:, : 6 * W], acc_hi[:, : 6 * W], acc_hi[:, 6 * W : 12 * W])
            nc.vector.tensor_add(acc_hi[:, : 3 * W], acc_hi[:, : 3 * W], acc_hi[:, 3 * W : 6 * W])
            # fold acc_lo [10, W] -> [W]
            nc.vector.tensor_add(acc_lo[:, : 5 * W], acc_lo[:, : 5 * W], acc_lo[:, 5 * W : 10 * W])
            for k in range(1, 5):
                nc.vector.tensor_add(acc_lo[:, :W], acc_lo[:, :W], acc_lo[:, k * W : (k + 1) * W])

            # ---- gap = <S_hi, g_hi> + <S_lo, g_low> -> SE MLP -> att ----
            nc.vector.tensor_mul(prod[:], acc_hi[:, : 3 * W], g_hi_t[:])
            nc.vector.tensor_reduce(
                partials[:, 0:1], prod[:], axis=mybir.AxisListType.X, op=add
            )
            nc.vector.tensor_mul(prod[:, :W], acc_lo[:, :W], g_low_t[:])
            nc.vector.tensor_reduce(
                partials[:, 1:2], prod[:, :W], axis=mybir.AxisListType.X, op=add
            )
            gap = small.tile([128, 1], f32, tag="gap")
            nc.vector.tensor_reduce(gap[:], partials[:], axis=mybir.AxisListType.X, op=add)
            y_p = psum.tile([C // 4, 1], f32, tag="y_p")
            nc.tensor.matmul(y_p[:], w1t_t[:], gap[:], start=True, stop=True)
            y_s = small.tile([C // 4, 1], f32, tag="y_s")
            nc.scalar.copy(y_s[:], y_p[:])
            z_p = psum.tile([C, 1], f32, tag="z_p")
            nc.tensor.matmul(z_p[:], w2t_t[:], y_s[:], start=True, stop=True)
            att = small.tile([128, 1], f32, tag="att")
            nc.scalar.activation(att[:], z_p[:], mybir.ActivationFunctionType.Sigmoid)

            # ---- pass 2: out = x * att ----
            # cached rows first (already on-chip)
            for xt, h0, rows in cached_tiles:
                nc.vector.tensor_scalar_mul(xt[:], xt[:], att[:, 0:1])
                nc.sync.dma_start(
                    out[:, h0 : h0 + rows, :].rearrange("p r w -> p (r w)"), xt[:]
                )
            # streamed rows
            for i, (h0, rows) in enumerate(all_tiles):
                if h0 >= CACHE_START:
                    continue
                xt = stream.tile([128, rows * W], f32, tag="xs")
                nc.sync.dma_start(
                    xt[:], x[:, h0 : h0 + rows, :].rearrange("p r w -> p (r w)")
                )
                if i % 3 == 2:
                    nc.scalar.mul(xt[:], xt[:], att[:, 0:1])
                else:
                    nc.vector.tensor_scalar_mul(xt[:], xt[:], att[:, 0:1])
                nc.sync.dma_start(
                    out[:, h0 : h0 + rows, :].rearrange("p r w -> p (r w)"), xt[:]
                )

    nc.compile()
    return nc


def _get_compiled():
    global _nc_cache, _const_cache
    if _nc_cache is None:
        _nc_cache = _build_kernel()
        _const_cache = _build_constants()
    return _nc_cache, _const_cache


def kernel(x, w1, w2, **_unused):
    from concourse.bass_utils import run_bass_kernel_spmd

    nc, (g_low_rep, g_hi_rep) = _get_compiled()
    x = np.ascontiguousarray(np.asarray(x), dtype=np.float32)
    w1t = np.ascontiguousarray(np.asarray(w1, dtype=np.float32).T)
    w2t = np.ascontiguousarray(np.asarray(w2, dtype=np.float32).T)
    in_maps = [
        {
            "x": np.ascontiguousarray(x[i]),
            "g_low": g_low_rep,
            "g_hi": g_hi_rep,
            "w1t": w1t,
            "w2t": w2t,
        }
        for i in range(N_CORES)
    ]
    res = run_bass_kernel_spmd(nc, in_maps, list(range(N_CORES)))
    outs = [np.asarray(r["out"], dtype=np.float32) for r in res.results]
    return np.stack(outs, axis=0)


# revision 8
# speedup vs baseline: 1.0261x; 1.0261x over previous
"""Trainium2 Bass kernel for BalancedFrequencyAttention.

Math: the reference does DCT(W) -> frequency split/mix -> IDCT -> GAP -> tiny
SE gate -> x * att.  Everything from x to gap is linear, so gap[b,c] ==
sum_{h,n} x[b,c,h,n] * G[h,n] for a fixed matrix G.  G turns out to have only
4 distinct rows: one pattern for h < 80 and a period-3 family for h >= 80.
So the kernel is a memory-bound two-pass streaming kernel:
  pass 1: gap[c] = sum over tiles of reduce(x_tile * G_tile)   (VectorE TTR)
  MLP:    att = sigmoid(w2 @ (w1 @ gap))                        (TensorE)
  pass 2: out = x * att[c]                                      (per-partition scale)
Sharding: batch b=8 -> one sample per NeuronCore, no cross-core communication.
"""

import sys

if "/opt/trn_rl_repo" not in sys.path:
    sys.path.insert(0, "/opt/trn_rl_repo")

import numpy as np

B, C, H, W = 8, 128, 200, 480
K = W // 2          # 240: balanced-spectrum width
HS = H - W // 4     # 80: high-freq start row
N_CORES = 8

# ---- pass-1/2 tiling (rows of H) ----
LOW_TILES = [(h0, 10) for h0 in range(0, HS, 10)]        # 8 tiles, pattern period 1 row
HIGH_TILES = [(h0, 12) for h0 in range(HS, H, 12)]       # 10 tiles, pattern period 3 rows
N_CACHE = 5                                               # cache last 5 high tiles in SBUF
CACHE_START = H - N_CACHE * 12                            # rows >= 152 stay in SBUF

_nc_cache = None
_const_cache = None


def _build_constants():
    """Fold the whole DCT pipeline into per-row weight vectors (float64)."""

    def dct_mat(n):
        i = np.arange(n, dtype=np.float64)
        m = np.cos(np.pi * (2.0 * i[None, :] + 1.0) * i[:, None] / (2.0 * n)) * np.sqrt(2.0 / n)
        m[0] *= 1.0 / np.sqrt(2.0)
        return m

    D = dct_mat(W)
    D2 = dct_mat(K)
    s = D2.sum(axis=1)                     # row sums of the IDCT matrix
    t = s @ D[:K, :]                       # low-band weight profile, [W]
    alpha = 0.6 / (H * K)
    beta = 0.4 / (H * K)
    # high[b,c,h',k'] = x_dct[:, :, 80+f//400, 80+f%400], f = h'*240+k'; the
    # s-weight index (f mod 240) depends on h only through (h-80)%3.
    offs = [0, 160, 80]
    V = np.stack(
        [s[(offs[j] + np.arange(HS, W) - HS) % K] @ D[HS:W, :] for j in range(3)]
    )
    g_low = (alpha * t).astype(np.float32)                                  # [480]
    g_hi = np.concatenate([alpha * t + beta * V[j] for j in range(3)]).astype(np.float32)  # [1440]
    g_low_rep = np.ascontiguousarray(np.broadcast_to(g_low, (128, W)))
    g_hi_rep = np.ascontiguousarray(np.broadcast_to(g_hi, (128, 3 * W)))
    return g_low_rep, g_hi_rep


def _build_kernel():
    import concourse.bacc as bacc
    import concourse.tile as tile
    from concourse import mybir

    f32 = mybir.dt.float32
    nc = bacc.Bacc("TRN2", target_bir_lowering=False, debug=False, num_devices=N_CORES)

    x = nc.dram_tensor("x", [C, H, W], f32, kind="ExternalInput")
    g_low = nc.dram_tensor("g_low", [128, W], f32, kind="ExternalInput")
    g_hi = nc.dram_tensor("g_hi", [128, 3 * W], f32, kind="ExternalInput")
    w1t = nc.dram_tensor("w1t", [C, C // 4], f32, kind="ExternalInput")
    w2t = nc.dram_tensor("w2t", [C // 4, C], f32, kind="ExternalInput")
    out = nc.dram_tensor("out", [C, H, W], f32, kind="ExternalOutput")

    add = mybir.AluOpType.add
    mult = mybir.AluOpType.mult
    all_tiles = LOW_TILES + HIGH_TILES
    n_tiles = len(all_tiles)

    with tile.TileContext(nc) as tc:
        with (
            tc.tile_pool(name="stream", bufs=3) as stream,
            tc.tile_pool(name="cachep", bufs=N_CACHE) as cachep,
            tc.tile_pool(name="consts", bufs=1) as consts,
            tc.tile_pool(name="small", bufs=1) as small,
            tc.tile_pool(name="psum", bufs=1, space="PSUM") as psum,
        ):
            g_low_t = consts.tile([128, W], f32, tag="g_low")
            nc.sync.dma_start(g_low_t[:], g_low[:])
            g_hi_t = consts.tile([128, 3 * W], f32, tag="g_hi")
            nc.sync.dma_start(g_hi_t[:], g_hi[:])
            w1t_t = consts.tile([C, C // 4], f32, tag="w1t")
            nc.sync.dma_start(w1t_t[:], w1t[:])
            w2t_t = consts.tile([C // 4, C], f32, tag="w2t")
            nc.sync.dma_start(w2t_t[:], w2t[:])
            acc_hi = consts.tile([128, 3 * W], f32, tag="acc_hi")
            acc_lo = consts.tile([128, W], f32, tag="acc_lo")
            nc.vector.memset(acc_hi[:], 0.0)
            nc.vector.memset(acc_lo[:], 0.0)
            prod = consts.tile([128, 3 * W], f32, tag="prod")
            partials = small.tile([128, 2], f32, tag="partials")

            # ---- pass 1: stream x, accumulate h-class sums (fold per tile) ----
            cached_tiles = []
            for i, (h0, rows) in enumerate(all_tiles):
                cached = h0 >= CACHE_START
                pool = cachep if cached else stream
                xt = pool.tile([128, rows * W], f32, tag="cache" if cached else "xs")
                nc.sync.dma_start(
                    xt[:], x[:, h0 : h0 + rows, :].rearrange("p r w -> p (r w)")
                )
                if h0 < HS:  # rows -> acc_lo, one add per row
                    for k in range(rows):
                        nc.vector.tensor_add(
                            acc_lo[:], acc_lo[:], xt[:, k * W : (k + 1) * W]
                        )
                else:  # 3-row groups -> acc_hi (class-aligned)
                    for k in range(rows // 3):
                        nc.vector.tensor_add(
                            acc_hi[:], acc_hi[:], xt[:, k * 3 * W : (k + 1) * 3 * W]
                        )
                if cached:
                    cached_tiles.append((xt, h0, rows))

            # ---- gap = <S_hi, g_hi> + <S_lo, g_low> -> SE MLP -> att ----
            nc.vector.tensor_mul(prod[:], acc_hi[:], g_hi_t[:])
            nc.vector.tensor_reduce(
                partials[:, 0:1], prod[:], axis=mybir.AxisListType.X, op=add
            )
            nc.vector.tensor_mul(prod[:, :W], acc_lo[:], g_low_t[:])
            nc.vector.tensor_reduce(
                partials[:, 1:2], prod[:, :W], axis=mybir.AxisListType.X, op=add
            )
            gap = small.tile([128, 1], f32, tag="gap")
            nc.vector.tensor_reduce(gap[:], partials[:], axis=mybir.AxisListType.X, op=add)
            y_p = psum.tile([C // 4, 1], f32, tag="y_p")
            nc.tensor.matmul(y_p[:], w1t_t[:], gap[:], start=True, stop=True)
            y_s = small.tile([C // 4, 1], f32, tag="y_s")
            nc.scalar.copy(y_s[:], y_p[:])
            z_p = psum.tile([C, 1], f32, tag="z_p")
            nc.tensor.matmul(z_p[:], w2t_t[:], y_s[:], start=True, stop=True)
            att = small.tile([128, 1], f32, tag="att")
            nc.scalar.activation(att[:], z_p[:], mybir.ActivationFunctionType.Sigmoid)

            # ---- pass 2: out = x * att ----
            # cached rows first (already on-chip)
            for xt, h0, rows in cached_tiles:
                nc.vector.tensor_scalar_mul(xt[:], xt[:], att[:, 0:1])
                nc.sync.dma_start(
                    out[:, h0 : h0 + rows, :].rearrange("p r w -> p (r w)"), xt[:]
                )
            # streamed rows
            for i, (h0, rows) in enumerate(all_tiles):
                if h0 >= CACHE_START:
                    continue
                xt = stream.tile([128, rows * W], f32, tag="xs")
                nc.sync.dma_start(
                    xt[:], x[:, h0 : h0 + rows, :].rearrange("p r w -> p (r w)")
                )
                if i % 3 == 2:
                    nc.scalar.mul(xt[:], xt[:], att[:, 0:1])
                else:
                    nc.vector.tensor_scalar_mul(xt[:], xt[:], att[:, 0:1])
                nc.sync.dma_start(
                    out[:, h0 : h0 + rows, :].rearrange("p r w -> p (r w)"), xt[:]
                )

    nc.compile()
    return nc


def _get_compiled():
    global _nc_cache, _const_cache
    if _nc_cache is None:
        _nc_cache = _build_kernel()
        _const_cache = _build_constants()
    return _nc_cache, _const_cache


def kernel(x, w1, w2, **_unused):
    from concourse.bass_utils import run_bass_kernel_spmd

    nc, (g_low_rep, g_hi_rep) = _get_compiled()
    x = np.ascontiguousarray(np.asarray(x), dtype=np.float32)
    w1t = np.ascontiguousarray(np.asarray(w1, dtype=np.float32).T)
    w2t = np.ascontiguousarray(np.asarray(w2, dtype=np.float32).T)
    in_maps = [
        {
            "x": np.ascontiguousarray(x[i]),
            "g_low": g_low_rep,
            "g_hi": g_hi_rep,
            "w1t": w1t,
            "w2t": w2t,
        }
        for i in range(N_CORES)
    ]
    res = run_bass_kernel_spmd(nc, in_maps, list(range(N_CORES)))
    outs = [np.asarray(r["out"], dtype=np.float32) for r in res.results]
    return np.stack(outs, axis=0)
